# revision 25
# baseline (speedup 1.0000x reference)
"""Trainium2 Bass kernel for a Bahdanau-attention GRU decoder step.

Reference computation (B=64, L=2048, H=U=1024, E=256, V=32000):
    c      = hidden @ w2 + b2                                  [B,U]
    score  = tanh(enc @ w1 + b1 + c[:,None,:]) @ v_w + v_b     [B,L,1]
    attn   = softmax(score, axis=1)
    ctx    = sum(attn * enc, axis=1)                           [B,H]
    xin    = concat([ctx, emb[x]], -1)                         [B,H+E]
    zi     = xin @ gru_k + gru_b       (h0 = 0 so r-gate and gru_rk drop out)
    state  = (1 - sigmoid(zi_z)) * tanh(zi_h)                  [B,U]
    logits = state @ out_w + out_b                             [B,V]

Sharding: data-parallel over batch (8 batches/core) for the attention +
GRU; the vocab projection is tensor-parallel (out_w column-sharded,
4000 cols/core) after an AllGather of the 64x1024 state.

Scores are bounded (|score| <= sum|v_w| + |v_b| ~= 16.4 because of the
tanh), so softmax is computed without max-subtraction: p = exp(s),
w = p / sum(p).  This allows a single pass over enc_output.

The big matmul enc @ w1 contracts over H, so enc tiles are needed with
H on partitions: enc is loaded with a casting SWDGE DMA (f32 HBM ->
bf16 SBUF) and transposed on-chip with the DMA XBAR (2-byte dtype).
The matmul-heavy paths (scores, v-dot, context, GRU, vocab) all run in
bf16 with fp32 PSUM accumulation; small one-off matmuls run fp32.
Attention weights themselves are computed and written in fp32.
"""

import numpy as np
from contextlib import ExitStack

import concourse.bass as bass
import concourse.tile as tile
from concourse import bacc, mybir
from concourse.bass_utils import run_bass_kernel_spmd

F32 = mybir.dt.float32
BF16 = mybir.dt.bfloat16
AF = mybir.ActivationFunctionType

N_CORES = 8
B, L, H, U, E, V = 64, 2048, 1024, 1024, 256, 32000
B_SH = B // N_CORES          # batches per core
V_SH = V // N_CORES          # vocab columns per core


def build(n_cores=N_CORES, b_sh=B_SH, l=L, v_sh=V_SH):
    """Emit and compile the Bass program.  Returns the Bacc instance."""
    KE = (H + E) // 128          # 10 k-blocks for the GRU matmul
    LCH = l // 512               # 512-wide l-chunks per batch
    VCH = v_sh // 500            # 500-wide logit chunks per core
    assert l % 512 == 0 and v_sh % 500 == 0

    nc = bacc.Bacc(
        "TRN2", target_bir_lowering=False, debug=False, num_devices=n_cores
    )

    # ---- I/O ----
    enc = nc.dram_tensor("enc", [b_sh, l, H], F32, kind="ExternalInput").ap()
    hidT = nc.dram_tensor("hidT", [U, b_sh], F32, kind="ExternalInput").ap()
    xeT = nc.dram_tensor("xeT", [E, b_sh], F32, kind="ExternalInput").ap()
    w1 = nc.dram_tensor("w1", [H, U], F32, kind="ExternalInput").ap()
    w2 = nc.dram_tensor("w2", [U, U], F32, kind="ExternalInput").ap()
    b12 = nc.dram_tensor("b12", [U], F32, kind="ExternalInput").ap()
    vw = nc.dram_tensor("vw", [U, 1], F32, kind="ExternalInput").ap()
    vb = nc.dram_tensor("vb", [1, 1], F32, kind="ExternalInput").ap()
    gk = nc.dram_tensor("gk", [H + E, 2 * U], F32, kind="ExternalInput").ap()
    gb = nc.dram_tensor("gb", [2 * U], F32, kind="ExternalInput").ap()
    ow = nc.dram_tensor("ow", [U, v_sh], F32, kind="ExternalInput").ap()
    ob = nc.dram_tensor("ob", [1, v_sh], F32, kind="ExternalInput").ap()

    logits_o = nc.dram_tensor(
        "logits", [b_sh * n_cores, v_sh], F32, kind="ExternalOutput"
    ).ap()
    stateT_o = nc.dram_tensor("stateT", [U, b_sh], F32, kind="ExternalOutput").ap()
    attn_o = nc.dram_tensor("attn", [b_sh, l], F32, kind="ExternalOutput").ap()

    with tile.TileContext(nc) as tc, ExitStack() as ctx:
        singles = ctx.enter_context(tc.tile_pool(name="singles", bufs=1))

        # ---- persistent small tensors ----
        w1_bf = singles.tile([128, 8, U], BF16)
        v_bf = singles.tile([128, 8, 1], BF16)
        vb_sb = singles.tile([1, 1], F32)
        b12_sb = singles.tile([128, 8], F32)
        gb_sb = singles.tile([128, 16], F32)
        ones_row = singles.tile([1, 128], F32)
        hidT_sb = singles.tile([128, 8, b_sh], F32)
        bias_cT = singles.tile([128, 8, b_sh], F32)
        xinT_bf = singles.tile([128, KE, b_sh], BF16)
        stT_sb = singles.tile([128, 8, b_sh], F32)
        stTf_bf = singles.tile([128, 8, n_cores * b_sh], BF16)

        # casting loads (f32 dram -> bf16 sbuf) go on the SWDGE ring
        nc.gpsimd.dma_start(out=w1_bf, in_=w1.rearrange("(k p) u -> p k u", p=128))
        nc.gpsimd.dma_start(out=v_bf, in_=vw.rearrange("(k p) o -> p k o", p=128))
        nc.gpsimd.dma_start(out=xinT_bf[:, 8:KE, :],
                            in_=xeT.rearrange("(k p) b -> p k b", p=128))
        nc.sync.dma_start(out=vb_sb, in_=vb)
        nc.sync.dma_start(out=b12_sb, in_=b12.rearrange("(k p) -> p k", p=128))
        nc.sync.dma_start(out=gb_sb, in_=gb.rearrange("(k p) -> p k", p=128))
        nc.sync.dma_start(out=hidT_sb, in_=hidT.rearrange("(k p) b -> p k b", p=128))
        nc.vector.memset(ones_row, 1.0)

        # GRU weights (bf16) persist from mid-attention through the GRU phase;
        # their casting loads are emitted interleaved with the chunk loop.
        gkbf_p = ctx.enter_context(tc.tile_pool(name="gkbf", bufs=KE))
        gkts = []
        for _ in range(KE):
            gkt = gkbf_p.tile([128, 2 * U], BF16, tag="gkt")
            gkts.append(gkt)

        # ---- init: c_T = w2.T @ hidden.T + b1 + b2 (fp32, tiny) ----
        with tc.tile_pool(name="init_sb", bufs=2) as initp, \
             tc.tile_pool(name="init_ps", bufs=2, space="PSUM") as initps:
            for ub in range(8):
                w2u = initp.tile([128, 8, 128], F32, tag="w2u")
                nc.sync.dma_start(
                    out=w2u,
                    in_=w2[:, ub * 128:(ub + 1) * 128].rearrange(
                        "(k p) u -> p k u", p=128
                    ),
                )
                cps = initps.tile([128, b_sh], F32)
                for k in range(8):
                    nc.tensor.matmul(
                        cps, lhsT=w2u[:, k, :], rhs=hidT_sb[:, k, :],
                        start=(k == 0), stop=(k == 7),
                    )
                nc.scalar.activation(
                    out=bias_cT[:, ub, :], in_=cps,
                    func=AF.Identity, bias=b12_sb[:, ub:ub + 1],
                )

        # ---- attention ----
        with tc.tile_pool(name="ebf", bufs=4) as ebf_p, \
             tc.tile_pool(name="eT", bufs=3) as eT_p, \
             tc.tile_pool(name="tanh", bufs=4) as tanh_p, \
             tc.tile_pool(name="prow", bufs=2) as prow_p, \
             tc.tile_pool(name="small", bufs=3) as small_p, \
             tc.tile_pool(name="tt_ps", bufs=2, space="PSUM") as tt_ps, \
             tc.tile_pool(name="sc_ps", bufs=2, space="PSUM") as sc_ps, \
             tc.tile_pool(name="ctx_ps", bufs=1, space="PSUM") as ctx_ps:

            chunk_no = 0
            gk_loaded = 0
            for b in range(b_sh):
                ctx_psum = ctx_ps.tile([1, H], F32)
                p_buf = prow_p.tile([1, l], F32)
                zparts = small_p.tile([1, LCH], F32)

                for ch0 in range(0, LCH, 1):
                    g = min(1, LCH - ch0)   # chunks processed per pass
                    # spread the GRU-weight prefetch over the chunk loop,
                    # starting late enough not to starve the first chunks
                    if chunk_no % 2 == 0 and chunk_no >= 6 \
                            and (chunk_no - 6) // 2 < KE:
                        k = (chunk_no - 6) // 2
                        nc.gpsimd.dma_start(
                            out=gkts[k],
                            in_=gk.rearrange("(k p) n -> p k n", p=128)[:, k, :],
                        )
                        gk_loaded = k + 1
                    chunk_no += g

                    # one casting DMA + one xbar transpose per 512-l chunk
                    ebs, eTs, scs = [], [], []
                    for c in range(g):
                        ch = ch0 + c
                        eb = ebf_p.tile([128, 4, H], BF16, tag="eb")
                        nc.gpsimd.dma_start(
                            out=eb,
                            in_=enc[b, ch * 512:(ch + 1) * 512, :].rearrange(
                                "(j p) h -> p j h", p=128
                            ),
                        )
                        ebs.append(eb)
                        # eT_t[p, j, hb, x] = enc_chunk.T[hb*128+p, j*128+x]
                        eT_t = eT_p.tile([128, 4, 8, 128], BF16, tag="eT")
                        nc.sync.dma_start_transpose(
                            out=eT_t.rearrange("p j h x -> p (j h) x"),
                            in_=eb.rearrange("p j h -> p (j h)"),
                        )
                        eTs.append(eT_t)
                        sc = sc_ps.tile([1, 512], F32, tag="sc")
                        scs.append(sc)

                    for ub in range(8):
                        # consecutive matmuls share the stationary operand so
                        # the weight load amortizes over both chunks
                        tt = tt_ps.tile([128, g, 512], F32)
                        for hb in range(8):
                            for c in range(g):
                                nc.tensor.matmul(
                                    tt[:, c, :],
                                    lhsT=w1_bf[:, hb, ub * 128:(ub + 1) * 128],
                                    rhs=eTs[c][:, :, hb, :],
                                    start=(hb == 0),
                                    stop=(hb == 7),
                                )
                        for c in range(g):
                            th = tanh_p.tile([128, 512], BF16)
                            nc.scalar.activation(
                                out=th, in_=tt[:, c, :], func=AF.Tanh,
                                bias=bias_cT[:, ub, b:b + 1],
                            )
                            nc.tensor.matmul(
                                scs[c], lhsT=v_bf[:, ub, :], rhs=th,
                                start=(ub == 0), stop=(ub == 7),
                            )

                    for c in range(g):
                        ch = ch0 + c
                        nc.scalar.activation(
                            out=p_buf[:, ch * 512:(ch + 1) * 512], in_=scs[c],
                            func=AF.Exp, bias=vb_sb,
                            accum_out=zparts[:, ch:ch + 1],
                        )
                        # p columns for the ctx matmul: scatter rows across
                        # partitions with tiny DMAs, then downcast on DVE
                        pcf = small_p.tile([128, 4], F32, tag="pcf")
                        for j in range(4):
                            nc.sync.dma_start(
                                out=pcf[:, j:j + 1],
                                in_=p_buf[:, ch * 512 + j * 128:
                                          ch * 512 + (j + 1) * 128].rearrange(
                                    "o (p x) -> o p x", p=128
                                ),
                            )
                        pcb = small_p.tile([128, 4], BF16, tag="pcb")
                        nc.vector.tensor_copy(out=pcb, in_=pcf)
                        for j in range(4):
                            lb = ch * 4 + j
                            for hf in range(2):
                                nc.tensor.matmul(
                                    ctx_psum[:, hf * 512:(hf + 1) * 512],
                                    lhsT=pcb[:, j:j + 1],
                                    rhs=ebs[c][:, j, hf * 512:(hf + 1) * 512],
                                    start=(lb == 0),
                                    stop=(lb == 4 * LCH - 1),
                                )

                # ---- batch epilogue: Z, attention weights, ctx -> xin ----
                zsum = small_p.tile([1, 1], F32)
                nc.vector.tensor_reduce(
                    zsum, zparts, axis=mybir.AxisListType.X, op=mybir.AluOpType.add
                )
                rz = small_p.tile([1, 1], F32)
                nc.vector.reciprocal(out=rz, in_=zsum)
                wbuf = prow_p.tile([1, l], F32, tag="wbuf")
                nc.vector.tensor_scalar_mul(wbuf, p_buf, rz)
                nc.sync.dma_start(out=attn_o[b:b + 1, :], in_=wbuf)

                ctxsc = small_p.tile([1, H], F32, tag="ctxsc")
                nc.vector.tensor_scalar_mul(ctxsc, ctx_psum, rz)
                ctxc = small_p.tile([128, 8], F32, tag="ctxc")
                for k in range(8):
                    nc.sync.dma_start(
                        out=ctxc[:, k:k + 1],
                        in_=ctxsc[:, k * 128:(k + 1) * 128].rearrange(
                            "o (p x) -> o p x", p=128
                        ),
                    )
                nc.vector.tensor_copy(out=xinT_bf[:, 0:8, b], in_=ctxc)

        # any GRU-weight loads not covered by the interleaved prefetch
        for k in range(gk_loaded, KE):
            nc.gpsimd.dma_start(
                out=gkts[k],
                in_=gk.rearrange("(k p) n -> p k n", p=128)[:, k, :],
            )

        # ---- GRU (bf16 matmuls; weights already resident) ----
        with tc.tile_pool(name="gru_sb", bufs=1) as gru_p, \
             tc.tile_pool(name="gru_ps", bufs=2, space="PSUM") as gru_ps:
            z_sb = gru_p.tile([128, 8, b_sh], F32)
            hh_sb = gru_p.tile([128, 8, b_sh], F32)
            for ub in range(16):
                zi = gru_ps.tile([128, b_sh], F32)
                for k in range(KE):
                    nc.tensor.matmul(
                        zi,
                        lhsT=gkts[k][:, ub * 128:(ub + 1) * 128],
                        rhs=xinT_bf[:, k, :],
                        start=(k == 0),
                        stop=(k == KE - 1),
                    )
                if ub < 8:
                    nc.scalar.activation(
                        out=z_sb[:, ub, :], in_=zi,
                        func=AF.Sigmoid, bias=gb_sb[:, ub:ub + 1],
                    )
                else:
                    nc.scalar.activation(
                        out=hh_sb[:, ub - 8, :], in_=zi,
                        func=AF.Tanh, bias=gb_sb[:, ub:ub + 1],
                    )
            omz = gru_p.tile([128, 8, b_sh], F32)
            nc.vector.tensor_scalar(
                out=omz, in0=z_sb, scalar1=-1.0, scalar2=1.0,
                op0=mybir.AluOpType.mult, op1=mybir.AluOpType.add,
            )
            nc.vector.tensor_mul(out=stT_sb, in0=omz, in1=hh_sb)
            nc.sync.dma_start(
                out=stateT_o.rearrange("(k p) b -> p k b", p=128), in_=stT_sb
            )

        # ---- AllGather state, then vocab projection ----
        with tc.tile_pool(name="dram", bufs=1, space="DRAM") as dram_p:
            st_loc = dram_p.tile([U, b_sh], F32)
            st_all = dram_p.tile([n_cores, U, b_sh], F32)
            nc.sync.dma_start(
                out=st_loc.rearrange("(k p) b -> p k b", p=128), in_=stT_sb
            )
            if n_cores > 1:
                nc.gpsimd.collective_compute(
                    "AllGather",
                    mybir.AluOpType.bypass,
                    replica_groups=[list(range(n_cores))],
                    ins=[st_loc.opt()],
                    outs=[st_all.opt()],
                )
                for c in range(n_cores):
                    nc.gpsimd.dma_start(
                        out=stTf_bf[:, :, c * b_sh:(c + 1) * b_sh],
                        in_=st_all[c].rearrange("(k p) b -> p k b", p=128),
                    )
            else:
                nc.gpsimd.dma_start(
                    out=stTf_bf,
                    in_=st_loc.rearrange("(k p) b -> p k b", p=128),
                )

            with tc.tile_pool(name="owf", bufs=6) as owf_p, \
                 tc.tile_pool(name="owbf", bufs=12) as owbf_p, \
                 tc.tile_pool(name="lg_sb", bufs=2) as lg_sb, \
                 tc.tile_pool(name="lg_ps", bufs=2, space="PSUM") as lg_ps:
                nb = n_cores * b_sh
                for chv in range(VCH):
                    obt = lg_sb.tile([1, 500], F32, tag="obt")
                    nc.sync.dma_start(out=obt, in_=ob[:, chv * 500:(chv + 1) * 500])
                    lg = lg_ps.tile([nb, 500], F32)
                    nc.tensor.matmul(
                        lg, lhsT=ones_row[:, :nb], rhs=obt,
                        start=True, stop=False,
                    )
                    for k in range(8):
                        src = ow.rearrange("(k p) v -> p k v", p=128)[
                            :, k, chv * 500:(chv + 1) * 500
                        ]
                        if (chv * 8 + k) % 2 == 0:
                            # casting load on the SWDGE ring
                            owt = owbf_p.tile([128, 500], BF16, tag="owg")
                            nc.gpsimd.dma_start(out=owt, in_=src)
                        else:
                            # f32 load on the ACT HWDGE ring + DVE downcast
                            owf = owf_p.tile([128, 500], F32)
                            nc.scalar.dma_start(out=owf, in_=src)
                            owt = owbf_p.tile([128, 500], BF16, tag="owc")
                            nc.vector.tensor_copy(out=owt, in_=owf)
                        nc.tensor.matmul(
                            lg, lhsT=stTf_bf[:, k, :], rhs=owt,
                            start=False, stop=(k == 7),
                        )
                    lgs = lg_sb.tile([nb, 500], F32)
                    nc.vector.tensor_copy(out=lgs, in_=lg)
                    nc.sync.dma_start(
                        out=logits_o[:, chv * 500:(chv + 1) * 500], in_=lgs
                    )

    nc.compile()
    return nc


_CACHE = {}


def _compiled(key="full", **kw):
    if key not in _CACHE:
        _CACHE[key] = build(**kw)
    return _CACHE[key]


def make_in_maps(x, hidden, enc_output, emb, w1, b1, w2, b2, v_w, v_b,
                 gru_k, gru_rk, gru_b, out_w, out_b,
                 n_cores=N_CORES, b_sh=B_SH, v_sh=V_SH):
    """Host-side sharding of the full inputs into per-core input maps."""
    f = np.ascontiguousarray
    x = np.asarray(x).astype(np.int64)
    xe = np.asarray(emb)[x[:, 0]]                       # [B, E] token embeddings
    u = np.asarray(w1).shape[1]
    gk_zh = np.concatenate(
        [np.asarray(gru_k)[:, :u], np.asarray(gru_k)[:, 2 * u:3 * u]], axis=1
    )
    gb_zh = np.concatenate(
        [np.asarray(gru_b)[:u], np.asarray(gru_b)[2 * u:3 * u]]
    )
    b12v = np.asarray(b1) + np.asarray(b2)
    in_maps = []
    for c in range(n_cores):
        bs = slice(c * b_sh, (c + 1) * b_sh)
        vs = slice(c * v_sh, (c + 1) * v_sh)
        in_maps.append({
            "enc": f(np.asarray(enc_output)[bs]),
            "hidT": f(np.asarray(hidden)[bs].T),
            "xeT": f(xe[bs].T),
            "w1": f(np.asarray(w1)),
            "w2": f(np.asarray(w2)),
            "b12": f(b12v),
            "vw": f(np.asarray(v_w)),
            "vb": f(np.asarray(v_b).reshape(1, 1)),
            "gk": f(gk_zh),
            "gb": f(gb_zh),
            "ow": f(np.asarray(out_w)[:, vs]),
            "ob": f(np.asarray(out_b)[vs].reshape(1, -1)),
        })
    return in_maps


def assemble(results, n_cores=N_CORES):
    """Gather per-core results into the full (logits, state, attn) tuple."""
    logits = np.concatenate([r["logits"] for r in results], axis=1)
    state = np.concatenate([r["stateT"].T for r in results], axis=0)
    attn = np.concatenate([r["attn"] for r in results], axis=0)[:, :, None]
    return (logits.astype(np.float32), state.astype(np.float32),
            attn.astype(np.float32))


def kernel(x, hidden, enc_output, emb, w1, b1, w2, b2, v_w, v_b,
           gru_k, gru_rk, gru_b, out_w, out_b):
    nc = _compiled("full")
    in_maps = make_in_maps(x, hidden, enc_output, emb, w1, b1, w2, b2,
                           v_w, v_b, gru_k, gru_rk, gru_b, out_w, out_b)
    res = run_bass_kernel_spmd(nc, in_maps, list(range(N_CORES)))
    return assemble(res.results)


# revision 26
# speedup vs baseline: 1.0248x; 1.0248x over previous
"""Trainium2 Bass kernel for a Bahdanau-attention GRU decoder step.

Reference computation (B=64, L=2048, H=U=1024, E=256, V=32000):
    c      = hidden @ w2 + b2                                  [B,U]
    score  = tanh(enc @ w1 + b1 + c[:,None,:]) @ v_w + v_b     [B,L,1]
    attn   = softmax(score, axis=1)
    ctx    = sum(attn * enc, axis=1)                           [B,H]
    xin    = concat([ctx, emb[x]], -1)                         [B,H+E]
    zi     = xin @ gru_k + gru_b       (h0 = 0 so r-gate and gru_rk drop out)
    state  = (1 - sigmoid(zi_z)) * tanh(zi_h)                  [B,U]
    logits = state @ out_w + out_b                             [B,V]

Sharding: data-parallel over batch (8 batches/core) for the attention +
GRU; the vocab projection is tensor-parallel (out_w column-sharded,
4000 cols/core) after an AllGather of the 64x1024 state.

Scores are bounded (|score| <= sum|v_w| + |v_b| ~= 16.4 because of the
tanh), so softmax is computed without max-subtraction: p = exp(s),
w = p / sum(p).  This allows a single pass over enc_output.

The big matmul enc @ w1 contracts over H, so enc tiles are needed with
H on partitions: enc is loaded with a casting SWDGE DMA (f32 HBM ->
bf16 SBUF) and transposed on-chip with the DMA XBAR (2-byte dtype).
The matmul-heavy paths (scores, v-dot, context, GRU, vocab) all run in
bf16 with fp32 PSUM accumulation; small one-off matmuls run fp32.
Attention weights themselves are computed and written in fp32.
"""

import numpy as np
from contextlib import ExitStack

import concourse.bass as bass
import concourse.tile as tile
from concourse import bacc, mybir
from concourse.bass_utils import run_bass_kernel_spmd

F32 = mybir.dt.float32
BF16 = mybir.dt.bfloat16
AF = mybir.ActivationFunctionType

N_CORES = 8
B, L, H, U, E, V = 64, 2048, 1024, 1024, 256, 32000
B_SH = B // N_CORES          # batches per core
V_SH = V // N_CORES          # vocab columns per core


def build(n_cores=N_CORES, b_sh=B_SH, l=L, v_sh=V_SH):
    """Emit and compile the Bass program.  Returns the Bacc instance."""
    KE = (H + E) // 128          # 10 k-blocks for the GRU matmul
    LCH = l // 512               # 512-wide l-chunks per batch
    VCH = v_sh // 500            # 500-wide logit chunks per core
    assert l % 512 == 0 and v_sh % 500 == 0

    nc = bacc.Bacc(
        "TRN2", target_bir_lowering=False, debug=False, num_devices=n_cores
    )

    # ---- I/O ----
    enc = nc.dram_tensor("enc", [b_sh, l, H], F32, kind="ExternalInput").ap()
    hidT = nc.dram_tensor("hidT", [U, b_sh], F32, kind="ExternalInput").ap()
    xeT = nc.dram_tensor("xeT", [E, b_sh], F32, kind="ExternalInput").ap()
    w1 = nc.dram_tensor("w1", [H, U], F32, kind="ExternalInput").ap()
    w2 = nc.dram_tensor("w2", [U, U], F32, kind="ExternalInput").ap()
    b12 = nc.dram_tensor("b12", [U], F32, kind="ExternalInput").ap()
    vw = nc.dram_tensor("vw", [U, 1], F32, kind="ExternalInput").ap()
    vb = nc.dram_tensor("vb", [1, 1], F32, kind="ExternalInput").ap()
    gk = nc.dram_tensor("gk", [H + E, 2 * U], F32, kind="ExternalInput").ap()
    gb = nc.dram_tensor("gb", [2 * U], F32, kind="ExternalInput").ap()
    ow = nc.dram_tensor("ow", [U, v_sh], F32, kind="ExternalInput").ap()
    ob = nc.dram_tensor("ob", [1, v_sh], F32, kind="ExternalInput").ap()

    logits_o = nc.dram_tensor(
        "logits", [b_sh * n_cores, v_sh], F32, kind="ExternalOutput"
    ).ap()
    stateT_o = nc.dram_tensor("stateT", [U, b_sh], F32, kind="ExternalOutput").ap()
    attn_o = nc.dram_tensor("attn", [b_sh, l], F32, kind="ExternalOutput").ap()

    with tile.TileContext(nc) as tc, ExitStack() as ctx:
        singles = ctx.enter_context(tc.tile_pool(name="singles", bufs=1))

        # ---- persistent small tensors ----
        w1_bf = singles.tile([128, 8, U], BF16)
        v_bf = singles.tile([128, 8, 1], BF16)
        vb_sb = singles.tile([1, 1], F32)
        b12_sb = singles.tile([128, 8], F32)
        gb_sb = singles.tile([128, 16], F32)
        ones_row = singles.tile([1, 128], F32)
        hidT_sb = singles.tile([128, 8, b_sh], F32)
        bias_cT = singles.tile([128, 8, b_sh], F32)
        xinT_bf = singles.tile([128, KE, b_sh], BF16)
        stT_sb = singles.tile([128, 8, b_sh], F32)
        stTf_bf = singles.tile([128, 8, n_cores * b_sh], BF16)

        # casting loads (f32 dram -> bf16 sbuf) go on the SWDGE ring
        nc.gpsimd.dma_start(out=w1_bf, in_=w1.rearrange("(k p) u -> p k u", p=128))
        nc.gpsimd.dma_start(out=v_bf, in_=vw.rearrange("(k p) o -> p k o", p=128))
        nc.gpsimd.dma_start(out=xinT_bf[:, 8:KE, :],
                            in_=xeT.rearrange("(k p) b -> p k b", p=128))
        nc.sync.dma_start(out=vb_sb, in_=vb)
        nc.sync.dma_start(out=b12_sb, in_=b12.rearrange("(k p) -> p k", p=128))
        nc.sync.dma_start(out=gb_sb, in_=gb.rearrange("(k p) -> p k", p=128))
        nc.sync.dma_start(out=hidT_sb, in_=hidT.rearrange("(k p) b -> p k b", p=128))
        nc.vector.memset(ones_row, 1.0)

        # GRU weights (bf16) persist from mid-attention through the GRU phase;
        # their casting loads are emitted interleaved with the chunk loop.
        gkbf_p = ctx.enter_context(tc.tile_pool(name="gkbf", bufs=KE))
        gkts = []
        for _ in range(KE):
            gkt = gkbf_p.tile([128, 2 * U], BF16, tag="gkt")
            gkts.append(gkt)

        # ---- init: c_T = w2.T @ hidden.T + b1 + b2 (fp32, tiny) ----
        with tc.tile_pool(name="init_sb", bufs=2) as initp, \
             tc.tile_pool(name="init_ps", bufs=2, space="PSUM") as initps:
            for ub in range(8):
                w2u = initp.tile([128, 8, 128], F32, tag="w2u")
                nc.sync.dma_start(
                    out=w2u,
                    in_=w2[:, ub * 128:(ub + 1) * 128].rearrange(
                        "(k p) u -> p k u", p=128
                    ),
                )
                cps = initps.tile([128, b_sh], F32)
                for k in range(8):
                    nc.tensor.matmul(
                        cps, lhsT=w2u[:, k, :], rhs=hidT_sb[:, k, :],
                        start=(k == 0), stop=(k == 7),
                    )
                nc.scalar.activation(
                    out=bias_cT[:, ub, :], in_=cps,
                    func=AF.Identity, bias=b12_sb[:, ub:ub + 1],
                )

        # ---- attention ----
        with tc.tile_pool(name="ebf", bufs=4) as ebf_p, \
             tc.tile_pool(name="eT", bufs=3) as eT_p, \
             tc.tile_pool(name="tanh", bufs=4) as tanh_p, \
             tc.tile_pool(name="prow", bufs=2) as prow_p, \
             tc.tile_pool(name="small", bufs=3) as small_p, \
             tc.tile_pool(name="tt_ps", bufs=2, space="PSUM") as tt_ps, \
             tc.tile_pool(name="sc_ps", bufs=2, space="PSUM") as sc_ps, \
             tc.tile_pool(name="ctx_ps", bufs=1, space="PSUM") as ctx_ps:

            chunk_no = 0
            gk_loaded = 0
            for b in range(b_sh):
                ctx_psum = ctx_ps.tile([1, H], F32)
                p_buf = prow_p.tile([1, l], F32)
                zparts = small_p.tile([1, LCH], F32)

                for ch0 in range(0, LCH, 2):
                    g = min(2, LCH - ch0)   # chunks processed per pass
                    # spread the GRU-weight prefetch over the chunk loop,
                    # starting late enough not to starve the first chunks
                    if chunk_no % 2 == 0 and chunk_no >= 6 \
                            and (chunk_no - 6) // 2 < KE:
                        k = (chunk_no - 6) // 2
                        nc.gpsimd.dma_start(
                            out=gkts[k],
                            in_=gk.rearrange("(k p) n -> p k n", p=128)[:, k, :],
                        )
                        gk_loaded = k + 1
                    chunk_no += g

                    # one casting DMA + one xbar transpose per 512-l chunk
                    ebs, eTs, scs = [], [], []
                    for c in range(g):
                        ch = ch0 + c
                        eb = ebf_p.tile([128, 4, H], BF16, tag="eb")
                        nc.gpsimd.dma_start(
                            out=eb,
                            in_=enc[b, ch * 512:(ch + 1) * 512, :].rearrange(
                                "(j p) h -> p j h", p=128
                            ),
                        )
                        ebs.append(eb)
                        # eT_t[p, j, hb, x] = enc_chunk.T[hb*128+p, j*128+x]
                        eT_t = eT_p.tile([128, 4, 8, 128], BF16, tag="eT")
                        nc.sync.dma_start_transpose(
                            out=eT_t.rearrange("p j h x -> p (j h) x"),
                            in_=eb.rearrange("p j h -> p (j h)"),
                        )
                        eTs.append(eT_t)
                        sc = sc_ps.tile([1, 512], F32, tag="sc")
                        scs.append(sc)

                    for ub in range(8):
                        # consecutive matmuls share the stationary operand so
                        # the weight load amortizes over both chunks
                        tts = []
                        for c in range(g):
                            tt = tt_ps.tile([128, 512], F32, tag=f"tt{c}")
                            tts.append(tt)
                        for hb in range(8):
                            for c in range(g):
                                nc.tensor.matmul(
                                    tts[c],
                                    lhsT=w1_bf[:, hb, ub * 128:(ub + 1) * 128],
                                    rhs=eTs[c][:, :, hb, :],
                                    start=(hb == 0),
                                    stop=(hb == 7),
                                )
                        for c in range(g):
                            th = tanh_p.tile([128, 512], BF16)
                            nc.scalar.activation(
                                out=th, in_=tts[c], func=AF.Tanh,
                                bias=bias_cT[:, ub, b:b + 1],
                            )
                            nc.tensor.matmul(
                                scs[c], lhsT=v_bf[:, ub, :], rhs=th,
                                start=(ub == 0), stop=(ub == 7),
                            )

                    for c in range(g):
                        ch = ch0 + c
                        nc.scalar.activation(
                            out=p_buf[:, ch * 512:(ch + 1) * 512], in_=scs[c],
                            func=AF.Exp, bias=vb_sb,
                            accum_out=zparts[:, ch:ch + 1],
                        )
                        # p columns for the ctx matmul: scatter rows across
                        # partitions with tiny DMAs, then downcast on DVE
                        pcf = small_p.tile([128, 4], F32, tag="pcf")
                        for j in range(4):
                            nc.sync.dma_start(
                                out=pcf[:, j:j + 1],
                                in_=p_buf[:, ch * 512 + j * 128:
                                          ch * 512 + (j + 1) * 128].rearrange(
                                    "o (p x) -> o p x", p=128
                                ),
                            )
                        pcb = small_p.tile([128, 4], BF16, tag="pcb")
                        nc.vector.tensor_copy(out=pcb, in_=pcf)
                        for j in range(4):
                            lb = ch * 4 + j
                            for hf in range(2):
                                nc.tensor.matmul(
                                    ctx_psum[:, hf * 512:(hf + 1) * 512],
                                    lhsT=pcb[:, j:j + 1],
                                    rhs=ebs[c][:, j, hf * 512:(hf + 1) * 512],
                                    start=(lb == 0),
                                    stop=(lb == 4 * LCH - 1),
                                )

                # ---- batch epilogue: Z, attention weights, ctx -> xin ----
                zsum = small_p.tile([1, 1], F32)
                nc.vector.tensor_reduce(
                    zsum, zparts, axis=mybir.AxisListType.X, op=mybir.AluOpType.add
                )
                rz = small_p.tile([1, 1], F32)
                nc.vector.reciprocal(out=rz, in_=zsum)
                wbuf = prow_p.tile([1, l], F32, tag="wbuf")
                nc.vector.tensor_scalar_mul(wbuf, p_buf, rz)
                nc.sync.dma_start(out=attn_o[b:b + 1, :], in_=wbuf)

                ctxsc = small_p.tile([1, H], F32, tag="ctxsc")
                nc.vector.tensor_scalar_mul(ctxsc, ctx_psum, rz)
                ctxc = small_p.tile([128, 8], F32, tag="ctxc")
                for k in range(8):
                    nc.sync.dma_start(
                        out=ctxc[:, k:k + 1],
                        in_=ctxsc[:, k * 128:(k + 1) * 128].rearrange(
                            "o (p x) -> o p x", p=128
                        ),
                    )
                nc.vector.tensor_copy(out=xinT_bf[:, 0:8, b], in_=ctxc)

        # any GRU-weight loads not covered by the interleaved prefetch
        for k in range(gk_loaded, KE):
            nc.gpsimd.dma_start(
                out=gkts[k],
                in_=gk.rearrange("(k p) n -> p k n", p=128)[:, k, :],
            )

        # ---- GRU (bf16 matmuls; weights already resident) ----
        with tc.tile_pool(name="gru_sb", bufs=1) as gru_p, \
             tc.tile_pool(name="gru_ps", bufs=2, space="PSUM") as gru_ps:
            z_sb = gru_p.tile([128, 8, b_sh], F32)
            hh_sb = gru_p.tile([128, 8, b_sh], F32)
            for ub in range(16):
                zi = gru_ps.tile([128, b_sh], F32)
                for k in range(KE):
                    nc.tensor.matmul(
                        zi,
                        lhsT=gkts[k][:, ub * 128:(ub + 1) * 128],
                        rhs=xinT_bf[:, k, :],
                        start=(k == 0),
                        stop=(k == KE - 1),
                    )
                if ub < 8:
                    nc.scalar.activation(
                        out=z_sb[:, ub, :], in_=zi,
                        func=AF.Sigmoid, bias=gb_sb[:, ub:ub + 1],
                    )
                else:
                    nc.scalar.activation(
                        out=hh_sb[:, ub - 8, :], in_=zi,
                        func=AF.Tanh, bias=gb_sb[:, ub:ub + 1],
                    )
            omz = gru_p.tile([128, 8, b_sh], F32)
            nc.vector.tensor_scalar(
                out=omz, in0=z_sb, scalar1=-1.0, scalar2=1.0,
                op0=mybir.AluOpType.mult, op1=mybir.AluOpType.add,
            )
            nc.vector.tensor_mul(out=stT_sb, in0=omz, in1=hh_sb)
            nc.sync.dma_start(
                out=stateT_o.rearrange("(k p) b -> p k b", p=128), in_=stT_sb
            )

        # ---- AllGather state, then vocab projection ----
        with tc.tile_pool(name="dram", bufs=1, space="DRAM") as dram_p:
            st_loc = dram_p.tile([U, b_sh], F32)
            st_all = dram_p.tile([n_cores, U, b_sh], F32)
            nc.sync.dma_start(
                out=st_loc.rearrange("(k p) b -> p k b", p=128), in_=stT_sb
            )
            if n_cores > 1:
                nc.gpsimd.collective_compute(
                    "AllGather",
                    mybir.AluOpType.bypass,
                    replica_groups=[list(range(n_cores))],
                    ins=[st_loc.opt()],
                    outs=[st_all.opt()],
                )
                for c in range(n_cores):
                    nc.gpsimd.dma_start(
                        out=stTf_bf[:, :, c * b_sh:(c + 1) * b_sh],
                        in_=st_all[c].rearrange("(k p) b -> p k b", p=128),
                    )
            else:
                nc.gpsimd.dma_start(
                    out=stTf_bf,
                    in_=st_loc.rearrange("(k p) b -> p k b", p=128),
                )

            with tc.tile_pool(name="owf", bufs=6) as owf_p, \
                 tc.tile_pool(name="owbf", bufs=12) as owbf_p, \
                 tc.tile_pool(name="lg_sb", bufs=2) as lg_sb, \
                 tc.tile_pool(name="lg_ps", bufs=2, space="PSUM") as lg_ps:
                nb = n_cores * b_sh
                for chv in range(VCH):
                    obt = lg_sb.tile([1, 500], F32, tag="obt")
                    nc.sync.dma_start(out=obt, in_=ob[:, chv * 500:(chv + 1) * 500])
                    lg = lg_ps.tile([nb, 500], F32)
                    nc.tensor.matmul(
                        lg, lhsT=ones_row[:, :nb], rhs=obt,
                        start=True, stop=False,
                    )
                    for k in range(8):
                        src = ow.rearrange("(k p) v -> p k v", p=128)[
                            :, k, chv * 500:(chv + 1) * 500
                        ]
                        if (chv * 8 + k) % 2 == 0:
                            # casting load on the SWDGE ring
                            owt = owbf_p.tile([128, 500], BF16, tag="owg")
                            nc.gpsimd.dma_start(out=owt, in_=src)
                        else:
                            # f32 load on the ACT HWDGE ring + DVE downcast
                            owf = owf_p.tile([128, 500], F32)
                            nc.scalar.dma_start(out=owf, in_=src)
                            owt = owbf_p.tile([128, 500], BF16, tag="owc")
                            nc.vector.tensor_copy(out=owt, in_=owf)
                        nc.tensor.matmul(
                            lg, lhsT=stTf_bf[:, k, :], rhs=owt,
                            start=False, stop=(k == 7),
                        )
                    lgs = lg_sb.tile([nb, 500], F32)
                    nc.vector.tensor_copy(out=lgs, in_=lg)
                    nc.sync.dma_start(
                        out=logits_o[:, chv * 500:(chv + 1) * 500], in_=lgs
                    )

    nc.compile()
    return nc


_CACHE = {}


def _compiled(key="full", **kw):
    if key not in _CACHE:
        _CACHE[key] = build(**kw)
    return _CACHE[key]


def make_in_maps(x, hidden, enc_output, emb, w1, b1, w2, b2, v_w, v_b,
                 gru_k, gru_rk, gru_b, out_w, out_b,
                 n_cores=N_CORES, b_sh=B_SH, v_sh=V_SH):
    """Host-side sharding of the full inputs into per-core input maps."""
    f = np.ascontiguousarray
    x = np.asarray(x).astype(np.int64)
    xe = np.asarray(emb)[x[:, 0]]                       # [B, E] token embeddings
    u = np.asarray(w1).shape[1]
    gk_zh = np.concatenate(
        [np.asarray(gru_k)[:, :u], np.asarray(gru_k)[:, 2 * u:3 * u]], axis=1
    )
    gb_zh = np.concatenate(
        [np.asarray(gru_b)[:u], np.asarray(gru_b)[2 * u:3 * u]]
    )
    b12v = np.asarray(b1) + np.asarray(b2)
    in_maps = []
    for c in range(n_cores):
        bs = slice(c * b_sh, (c + 1) * b_sh)
        vs = slice(c * v_sh, (c + 1) * v_sh)
        in_maps.append({
            "enc": f(np.asarray(enc_output)[bs]),
            "hidT": f(np.asarray(hidden)[bs].T),
            "xeT": f(xe[bs].T),
            "w1": f(np.asarray(w1)),
            "w2": f(np.asarray(w2)),
            "b12": f(b12v),
            "vw": f(np.asarray(v_w)),
            "vb": f(np.asarray(v_b).reshape(1, 1)),
            "gk": f(gk_zh),
            "gb": f(gb_zh),
            "ow": f(np.asarray(out_w)[:, vs]),
            "ob": f(np.asarray(out_b)[vs].reshape(1, -1)),
        })
    return in_maps


def assemble(results, n_cores=N_CORES):
    """Gather per-core results into the full (logits, state, attn) tuple."""
    logits = np.concatenate([r["logits"] for r in results], axis=1)
    state = np.concatenate([r["stateT"].T for r in results], axis=0)
    attn = np.concatenate([r["attn"] for r in results], axis=0)[:, :, None]
    return (logits.astype(np.float32), state.astype(np.float32),
            attn.astype(np.float32))


def kernel(x, hidden, enc_output, emb, w1, b1, w2, b2, v_w, v_b,
           gru_k, gru_rk, gru_b, out_w, out_b):
    nc = _compiled("full")
    in_maps = make_in_maps(x, hidden, enc_output, emb, w1, b1, w2, b2,
                           v_w, v_b, gru_k, gru_rk, gru_b, out_w, out_b)
    res = run_bass_kernel_spmd(nc, in_maps, list(range(N_CORES)))
    return assemble(res.results)


# revision 27
# speedup vs baseline: 1.0305x; 1.0055x over previous
"""Trainium2 Bass kernel for a Bahdanau-attention GRU decoder step.

Reference computation (B=64, L=2048, H=U=1024, E=256, V=32000):
    c      = hidden @ w2 + b2                                  [B,U]
    score  = tanh(enc @ w1 + b1 + c[:,None,:]) @ v_w + v_b     [B,L,1]
    attn   = softmax(score, axis=1)
    ctx    = sum(attn * enc, axis=1)                           [B,H]
    xin    = concat([ctx, emb[x]], -1)                         [B,H+E]
    zi     = xin @ gru_k + gru_b       (h0 = 0 so r-gate and gru_rk drop out)
    state  = (1 - sigmoid(zi_z)) * tanh(zi_h)                  [B,U]
    logits = state @ out_w + out_b                             [B,V]

Sharding: data-parallel over batch (8 batches/core) for the attention +
GRU; the vocab projection is tensor-parallel (out_w column-sharded,
4000 cols/core) after an AllGather of the 64x1024 state.

Scores are bounded (|score| <= sum|v_w| + |v_b| ~= 16.4 because of the
tanh), so softmax is computed without max-subtraction: p = exp(s),
w = p / sum(p).  This allows a single pass over enc_output.

The big matmul enc @ w1 contracts over H, so enc tiles are needed with
H on partitions: enc is loaded with a casting SWDGE DMA (f32 HBM ->
bf16 SBUF) and transposed on-chip with the DMA XBAR (2-byte dtype).
The matmul-heavy paths (scores, v-dot, context, GRU, vocab) all run in
bf16 with fp32 PSUM accumulation; small one-off matmuls run fp32.
Attention weights themselves are computed and written in fp32.
"""

import numpy as np
from contextlib import ExitStack

import concourse.bass as bass
import concourse.tile as tile
from concourse import bacc, mybir
from concourse.bass_utils import run_bass_kernel_spmd

F32 = mybir.dt.float32
BF16 = mybir.dt.bfloat16
AF = mybir.ActivationFunctionType

N_CORES = 8
B, L, H, U, E, V = 64, 2048, 1024, 1024, 256, 32000
B_SH = B // N_CORES          # batches per core
V_SH = V // N_CORES          # vocab columns per core


def build(n_cores=N_CORES, b_sh=B_SH, l=L, v_sh=V_SH):
    """Emit and compile the Bass program.  Returns the Bacc instance."""
    KE = (H + E) // 128          # 10 k-blocks for the GRU matmul
    LCH = l // 512               # 512-wide l-chunks per batch
    VCH = v_sh // 500            # 500-wide logit chunks per core
    assert l % 512 == 0 and v_sh % 500 == 0

    nc = bacc.Bacc(
        "TRN2", target_bir_lowering=False, debug=False, num_devices=n_cores
    )

    # ---- I/O ----
    enc = nc.dram_tensor("enc", [b_sh, l, H], F32, kind="ExternalInput").ap()
    hidT = nc.dram_tensor("hidT", [U, b_sh], F32, kind="ExternalInput").ap()
    xeT = nc.dram_tensor("xeT", [E, b_sh], F32, kind="ExternalInput").ap()
    w1 = nc.dram_tensor("w1", [H, U], F32, kind="ExternalInput").ap()
    w2 = nc.dram_tensor("w2", [U, U], F32, kind="ExternalInput").ap()
    b12 = nc.dram_tensor("b12", [U], F32, kind="ExternalInput").ap()
    vw = nc.dram_tensor("vw", [U, 1], F32, kind="ExternalInput").ap()
    vb = nc.dram_tensor("vb", [1, 1], F32, kind="ExternalInput").ap()
    gk = nc.dram_tensor("gk", [H + E, 2 * U], F32, kind="ExternalInput").ap()
    gb = nc.dram_tensor("gb", [2 * U], F32, kind="ExternalInput").ap()
    ow = nc.dram_tensor("ow", [U, v_sh], F32, kind="ExternalInput").ap()
    ob = nc.dram_tensor("ob", [1, v_sh], F32, kind="ExternalInput").ap()

    logits_o = nc.dram_tensor(
        "logits", [b_sh * n_cores, v_sh], F32, kind="ExternalOutput"
    ).ap()
    stateT_o = nc.dram_tensor("stateT", [U, b_sh], F32, kind="ExternalOutput").ap()
    attn_o = nc.dram_tensor("attn", [b_sh, l], F32, kind="ExternalOutput").ap()

    with tile.TileContext(nc) as tc, ExitStack() as ctx:
        singles = ctx.enter_context(tc.tile_pool(name="singles", bufs=1))

        # ---- persistent small tensors ----
        w1_bf = singles.tile([128, 8, U], BF16)
        v_bf = singles.tile([128, 8, 1], BF16)
        vb_sb = singles.tile([1, 1], F32)
        b12_sb = singles.tile([128, 8], F32)
        gb_sb = singles.tile([128, 16], F32)
        ones_row = singles.tile([1, 128], F32)
        hidT_sb = singles.tile([128, 8, b_sh], F32)
        bias_cT = singles.tile([128, 8, b_sh], F32)
        xinT_bf = singles.tile([128, KE, b_sh], BF16)
        stT_sb = singles.tile([128, 8, b_sh], F32)
        stTf_bf = singles.tile([128, 8, n_cores * b_sh], BF16)

        # casting loads (f32 dram -> bf16 sbuf) go on the SWDGE ring
        nc.gpsimd.dma_start(out=w1_bf, in_=w1.rearrange("(k p) u -> p k u", p=128))
        nc.gpsimd.dma_start(out=v_bf, in_=vw.rearrange("(k p) o -> p k o", p=128))
        nc.gpsimd.dma_start(out=xinT_bf[:, 8:KE, :],
                            in_=xeT.rearrange("(k p) b -> p k b", p=128))
        nc.sync.dma_start(out=vb_sb, in_=vb)
        nc.sync.dma_start(out=b12_sb, in_=b12.rearrange("(k p) -> p k", p=128))
        nc.sync.dma_start(out=gb_sb, in_=gb.rearrange("(k p) -> p k", p=128))
        nc.sync.dma_start(out=hidT_sb, in_=hidT.rearrange("(k p) b -> p k b", p=128))
        nc.vector.memset(ones_row, 1.0)

        # GRU weights (bf16) persist from mid-attention through the GRU phase;
        # their casting loads are emitted interleaved with the chunk loop.
        gkbf_p = ctx.enter_context(tc.tile_pool(name="gkbf", bufs=KE))
        gkts = []
        for _ in range(KE):
            gkt = gkbf_p.tile([128, 2 * U], BF16, tag="gkt")
            gkts.append(gkt)

        # ---- init: c_T = w2.T @ hidden.T + b1 + b2 (fp32, tiny) ----
        with tc.tile_pool(name="init_sb", bufs=2) as initp, \
             tc.tile_pool(name="init_ps", bufs=2, space="PSUM") as initps:
            for ub in range(8):
                w2u = initp.tile([128, 8, 128], F32, tag="w2u")
                nc.sync.dma_start(
                    out=w2u,
                    in_=w2[:, ub * 128:(ub + 1) * 128].rearrange(
                        "(k p) u -> p k u", p=128
                    ),
                )
                cps = initps.tile([128, b_sh], F32)
                for k in range(8):
                    nc.tensor.matmul(
                        cps, lhsT=w2u[:, k, :], rhs=hidT_sb[:, k, :],
                        start=(k == 0), stop=(k == 7),
                    )
                nc.scalar.activation(
                    out=bias_cT[:, ub, :], in_=cps,
                    func=AF.Identity, bias=b12_sb[:, ub:ub + 1],
                )

        # ---- attention ----
        with tc.tile_pool(name="ebf", bufs=4) as ebf_p, \
             tc.tile_pool(name="eT", bufs=3) as eT_p, \
             tc.tile_pool(name="tanh", bufs=4) as tanh_p, \
             tc.tile_pool(name="prow", bufs=2) as prow_p, \
             tc.tile_pool(name="small", bufs=3) as small_p, \
             tc.tile_pool(name="tt_ps", bufs=2, space="PSUM") as tt_ps, \
             tc.tile_pool(name="sc_ps", bufs=2, space="PSUM") as sc_ps, \
             tc.tile_pool(name="ctx_ps", bufs=1, space="PSUM") as ctx_ps:

            chunk_no = 0
            gk_loaded = 0
            for b in range(b_sh):
                ctx_psum = ctx_ps.tile([1, H], F32)
                p_buf = prow_p.tile([1, l], F32)
                zparts = small_p.tile([1, LCH], F32)

                for ch0 in range(0, LCH, 2):
                    g = min(2, LCH - ch0)   # chunks processed per pass
                    # spread the GRU-weight prefetch over the chunk loop,
                    # starting late enough not to starve the first chunks
                    if chunk_no % 2 == 0 and chunk_no >= 6 \
                            and (chunk_no - 6) // 2 < KE:
                        k = (chunk_no - 6) // 2
                        nc.gpsimd.dma_start(
                            out=gkts[k],
                            in_=gk.rearrange("(k p) n -> p k n", p=128)[:, k, :],
                        )
                        gk_loaded = k + 1
                    chunk_no += g

                    # one casting DMA + one xbar transpose per 512-l chunk
                    ebs, eTs, scs = [], [], []
                    for c in range(g):
                        ch = ch0 + c
                        eb = ebf_p.tile([128, 4, H], BF16, tag="eb")
                        nc.gpsimd.dma_start(
                            out=eb,
                            in_=enc[b, ch * 512:(ch + 1) * 512, :].rearrange(
                                "(j p) h -> p j h", p=128
                            ),
                        )
                        ebs.append(eb)
                        # eT_t[p, j, hb, x] = enc_chunk.T[hb*128+p, j*128+x]
                        eT_t = eT_p.tile([128, 4, 8, 128], BF16, tag="eT")
                        nc.sync.dma_start_transpose(
                            out=eT_t.rearrange("p j h x -> p (j h) x"),
                            in_=eb.rearrange("p j h -> p (j h)"),
                        )
                        eTs.append(eT_t)
                        sc = sc_ps.tile([1, 512], F32, tag="sc")
                        scs.append(sc)

                    for ub in range(8):
                        # consecutive matmuls share the stationary operand so
                        # the weight load amortizes over both chunks
                        tts = []
                        for c in range(g):
                            tt = tt_ps.tile([128, 512], F32, tag=f"tt{c}")
                            tts.append(tt)
                        for hb in range(8):
                            for c in range(g):
                                nc.tensor.matmul(
                                    tts[c],
                                    lhsT=w1_bf[:, hb, ub * 128:(ub + 1) * 128],
                                    rhs=eTs[c][:, :, hb, :],
                                    start=(hb == 0),
                                    stop=(hb == 7),
                                )
                        for c in range(g):
                            th = tanh_p.tile([128, 512], BF16)
                            nc.scalar.activation(
                                out=th, in_=tts[c], func=AF.Tanh,
                                bias=bias_cT[:, ub, b:b + 1],
                            )
                            nc.tensor.matmul(
                                scs[c], lhsT=v_bf[:, ub, :], rhs=th,
                                start=(ub == 0), stop=(ub == 7),
                            )

                    for c in range(g):
                        ch = ch0 + c
                        nc.scalar.activation(
                            out=p_buf[:, ch * 512:(ch + 1) * 512], in_=scs[c],
                            func=AF.Exp, bias=vb_sb,
                            accum_out=zparts[:, ch:ch + 1],
                        )
                        # p columns for the ctx matmul: scatter rows across
                        # partitions with tiny DMAs, then downcast on DVE
                        pcf = small_p.tile([128, 4], F32, tag="pcf")
                        for j in range(4):
                            nc.sync.dma_start(
                                out=pcf[:, j:j + 1],
                                in_=p_buf[:, ch * 512 + j * 128:
                                          ch * 512 + (j + 1) * 128].rearrange(
                                    "o (p x) -> o p x", p=128
                                ),
                            )
                        pcb = small_p.tile([128, 4], BF16, tag="pcb")
                        nc.vector.tensor_copy(out=pcb, in_=pcf)
                        for j in range(4):
                            lb = ch * 4 + j
                            for hf in range(2):
                                nc.tensor.matmul(
                                    ctx_psum[:, hf * 512:(hf + 1) * 512],
                                    lhsT=pcb[:, j:j + 1],
                                    rhs=ebs[c][:, j, hf * 512:(hf + 1) * 512],
                                    start=(lb == 0),
                                    stop=(lb == 4 * LCH - 1),
                                )

                # ---- batch epilogue: Z, attention weights, ctx -> xin ----
                zsum = small_p.tile([1, 1], F32)
                nc.vector.tensor_reduce(
                    zsum, zparts, axis=mybir.AxisListType.X, op=mybir.AluOpType.add
                )
                rz = small_p.tile([1, 1], F32)
                nc.vector.reciprocal(out=rz, in_=zsum)
                wbuf = prow_p.tile([1, l], F32, tag="wbuf")
                nc.vector.tensor_scalar_mul(wbuf, p_buf, rz)
                nc.sync.dma_start(out=attn_o[b:b + 1, :], in_=wbuf)

                ctxsc = small_p.tile([1, H], F32, tag="ctxsc")
                nc.vector.tensor_scalar_mul(ctxsc, ctx_psum, rz)
                ctxc = small_p.tile([128, 8], F32, tag="ctxc")
                for k in range(8):
                    nc.sync.dma_start(
                        out=ctxc[:, k:k + 1],
                        in_=ctxsc[:, k * 128:(k + 1) * 128].rearrange(
                            "o (p x) -> o p x", p=128
                        ),
                    )
                nc.vector.tensor_copy(out=xinT_bf[:, 0:8, b], in_=ctxc)

        # any GRU-weight loads not covered by the interleaved prefetch
        for k in range(gk_loaded, KE):
            nc.gpsimd.dma_start(
                out=gkts[k],
                in_=gk.rearrange("(k p) n -> p k n", p=128)[:, k, :],
            )

        # ---- GRU (bf16 matmuls; weights already resident) ----
        with tc.tile_pool(name="gru_sb", bufs=1) as gru_p, \
             tc.tile_pool(name="gru_ps", bufs=2, space="PSUM") as gru_ps:
            z_sb = gru_p.tile([128, 8, b_sh], F32)
            hh_sb = gru_p.tile([128, 8, b_sh], F32)
            for ub in range(16):
                zi = gru_ps.tile([128, b_sh], F32)
                for k in range(KE):
                    nc.tensor.matmul(
                        zi,
                        lhsT=gkts[k][:, ub * 128:(ub + 1) * 128],
                        rhs=xinT_bf[:, k, :],
                        start=(k == 0),
                        stop=(k == KE - 1),
                    )
                if ub < 8:
                    nc.scalar.activation(
                        out=z_sb[:, ub, :], in_=zi,
                        func=AF.Sigmoid, bias=gb_sb[:, ub:ub + 1],
                    )
                else:
                    nc.scalar.activation(
                        out=hh_sb[:, ub - 8, :], in_=zi,
                        func=AF.Tanh, bias=gb_sb[:, ub:ub + 1],
                    )
            omz = gru_p.tile([128, 8, b_sh], F32)
            nc.vector.tensor_scalar(
                out=omz, in0=z_sb, scalar1=-1.0, scalar2=1.0,
                op0=mybir.AluOpType.mult, op1=mybir.AluOpType.add,
            )
            nc.vector.tensor_mul(out=stT_sb, in0=omz, in1=hh_sb)
            nc.sync.dma_start(
                out=stateT_o.rearrange("(k p) b -> p k b", p=128), in_=stT_sb
            )

        # ---- AllGather state, then vocab projection ----
        with tc.tile_pool(name="dram", bufs=1, space="DRAM") as dram_p:
            st_loc = dram_p.tile([U, b_sh], F32)
            st_all = dram_p.tile([n_cores, U, b_sh], F32)
            nc.sync.dma_start(
                out=st_loc.rearrange("(k p) b -> p k b", p=128), in_=stT_sb
            )
            if n_cores > 1:
                nc.gpsimd.collective_compute(
                    "AllGather",
                    mybir.AluOpType.bypass,
                    replica_groups=[list(range(n_cores))],
                    ins=[st_loc.opt()],
                    outs=[st_all.opt()],
                )
                for c in range(n_cores):
                    nc.gpsimd.dma_start(
                        out=stTf_bf[:, :, c * b_sh:(c + 1) * b_sh],
                        in_=st_all[c].rearrange("(k p) b -> p k b", p=128),
                    )
            else:
                nc.gpsimd.dma_start(
                    out=stTf_bf,
                    in_=st_loc.rearrange("(k p) b -> p k b", p=128),
                )

            with tc.tile_pool(name="owf", bufs=14) as owf_p, \
                 tc.tile_pool(name="owbf", bufs=24) as owbf_p, \
                 tc.tile_pool(name="lg_sb", bufs=2) as lg_sb, \
                 tc.tile_pool(name="lg_ps", bufs=2, space="PSUM") as lg_ps:
                nb = n_cores * b_sh
                for chv in range(VCH):
                    obt = lg_sb.tile([1, 500], F32, tag="obt")
                    nc.sync.dma_start(out=obt, in_=ob[:, chv * 500:(chv + 1) * 500])
                    lg = lg_ps.tile([nb, 500], F32)
                    nc.tensor.matmul(
                        lg, lhsT=ones_row[:, :nb], rhs=obt,
                        start=True, stop=False,
                    )
                    for k in range(8):
                        src = ow.rearrange("(k p) v -> p k v", p=128)[
                            :, k, chv * 500:(chv + 1) * 500
                        ]
                        if (chv * 8 + k) % 2 == 0:
                            # casting load on the SWDGE ring
                            owt = owbf_p.tile([128, 500], BF16, tag="owg")
                            nc.gpsimd.dma_start(out=owt, in_=src)
                        else:
                            # f32 load on the ACT HWDGE ring + DVE downcast
                            owf = owf_p.tile([128, 500], F32)
                            nc.scalar.dma_start(out=owf, in_=src)
                            owt = owbf_p.tile([128, 500], BF16, tag="owc")
                            nc.vector.tensor_copy(out=owt, in_=owf)
                        nc.tensor.matmul(
                            lg, lhsT=stTf_bf[:, k, :], rhs=owt,
                            start=False, stop=(k == 7),
                        )
                    lgs = lg_sb.tile([nb, 500], F32)
                    nc.vector.tensor_copy(out=lgs, in_=lg)
                    nc.sync.dma_start(
                        out=logits_o[:, chv * 500:(chv + 1) * 500], in_=lgs
                    )

    nc.compile()
    return nc


_CACHE = {}


def _compiled(key="full", **kw):
    if key not in _CACHE:
        _CACHE[key] = build(**kw)
    return _CACHE[key]


def make_in_maps(x, hidden, enc_output, emb, w1, b1, w2, b2, v_w, v_b,
                 gru_k, gru_rk, gru_b, out_w, out_b,
                 n_cores=N_CORES, b_sh=B_SH, v_sh=V_SH):
    """Host-side sharding of the full inputs into per-core input maps."""
    f = np.ascontiguousarray
    x = np.asarray(x).astype(np.int64)
    xe = np.asarray(emb)[x[:, 0]]                       # [B, E] token embeddings
    u = np.asarray(w1).shape[1]
    gk_zh = np.concatenate(
        [np.asarray(gru_k)[:, :u], np.asarray(gru_k)[:, 2 * u:3 * u]], axis=1
    )
    gb_zh = np.concatenate(
        [np.asarray(gru_b)[:u], np.asarray(gru_b)[2 * u:3 * u]]
    )
    b12v = np.asarray(b1) + np.asarray(b2)
    in_maps = []
    for c in range(n_cores):
        bs = slice(c * b_sh, (c + 1) * b_sh)
        vs = slice(c * v_sh, (c + 1) * v_sh)
        in_maps.append({
            "enc": f(np.asarray(enc_output)[bs]),
            "hidT": f(np.asarray(hidden)[bs].T),
            "xeT": f(xe[bs].T),
            "w1": f(np.asarray(w1)),
            "w2": f(np.asarray(w2)),
            "b12": f(b12v),
            "vw": f(np.asarray(v_w)),
            "vb": f(np.asarray(v_b).reshape(1, 1)),
            "gk": f(gk_zh),
            "gb": f(gb_zh),
            "ow": f(np.asarray(out_w)[:, vs]),
            "ob": f(np.asarray(out_b)[vs].reshape(1, -1)),
        })
    return in_maps


def assemble(results, n_cores=N_CORES):
    """Gather per-core results into the full (logits, state, attn) tuple."""
    logits = np.concatenate([r["logits"] for r in results], axis=1)
    state = np.concatenate([r["stateT"].T for r in results], axis=0)
    attn = np.concatenate([r["attn"] for r in results], axis=0)[:, :, None]
    return (logits.astype(np.float32), state.astype(np.float32),
            attn.astype(np.float32))


def kernel(x, hidden, enc_output, emb, w1, b1, w2, b2, v_w, v_b,
           gru_k, gru_rk, gru_b, out_w, out_b):
    nc = _compiled("full")
    in_maps = make_in_maps(x, hidden, enc_output, emb, w1, b1, w2, b2,
                           v_w, v_b, gru_k, gru_rk, gru_b, out_w, out_b)
    res = run_bass_kernel_spmd(nc, in_maps, list(range(N_CORES)))
    return assemble(res.results)
